# revision 1
# baseline (speedup 1.0000x reference)
"""DCT sequence-compression kernel for TRN2 (nn_CompressedModel).

For x [B=64, T=1024, D=768] fp32 computes (matching the reference):
  x_dct = (C_T @ x)[:, :k, :]          k = 922
  x_rec = C_k^T @ x_dct
returning (x_rec, x_dct).

Both outputs are linear in x along tokens. DCT mirror symmetry
C[k, T-1-t] = (-1)^k C[k, t] lets us fold x (host-side, pure data prep):
  e = x[:512] + rev(x[512:]),  o = x[:512] - rev(x[512:])
so even dct rows contract only e (512-long), odd rows only o, and the
reconstruction rows accumulate symmetric/antisymmetric weight halves in
PSUM — ~1.9x less tensor-engine streaming than the naive dual matmul.
Combined projection weights are built on the host; matmuls run in
float32r (full-rate fp32 PE mode, ~1.5e-4 rel err). Pure data parallel
over B across 8 cores. KERNEL_LEVEL=2 selects a second fold of e
(ee/eo); measured slower on HW despite fewer streamed columns (weight
reload overhead per matmul stops hiding), so level 1 is the default.
"""

import os

import numpy as np

# The trimmed axon environment has no NTFF profile hook; make sure
# run_bass_kernel_spmd never tries the trace path.
os.environ["BASS_NEVER_TRACE"] = "1"

import concourse.bass as bass  # noqa: F401
import concourse.mybir as mybir
import concourse.tile as tile
from concourse import bacc
from concourse.bass_utils import run_bass_kernel_spmd

B, T, D = 64, 1024, 768
K = 922              # ceil(0.9 * 1024)
KPAD = 928           # dct rows padded to a multiple of 4 on device
H = T // 2           # 512: o contraction length
Q = T // 4           # 256: ee/eo contraction length
NEE = 231            # rows k%4==0 (k<=920)
NEO = 230            # rows k%4==2 (k<=918)
NODD = 461           # odd rows
N_CORES = 8
BPC = B // N_CORES   # batches per core
P = 128
CCO = H // P         # 4 contraction chunks for o
CCE = Q // P         # 2 contraction chunks for ee/eo
N0 = 512             # first free-dim split (PSUM bank)

MM_DTYPE = mybir.dt.float32r


def _chunks(n, p=P):
    return [(i * p, min(p, n - i * p)) for i in range((n + p - 1) // p)]


EE_CHUNKS = _chunks(NEE)   # 2 chunks
EO_CHUNKS = _chunks(NEO)   # 2 chunks
O_CHUNKS = _chunks(NODD)   # 4 chunks
C_CHUNKS = _chunks(K)      # 8 chunks (reconstruction rows)


def _dct_matrix(N: int) -> np.ndarray:
    """Orthonormal DCT-II matrix [N, N] in float64."""
    n = np.arange(N, dtype=np.float64)
    C = np.cos(np.pi * (2.0 * n[None, :] + 1.0) * n[:, None] / (2.0 * N))
    s = np.full(N, np.sqrt(2.0 / N))
    s[0] = np.sqrt(1.0 / N)
    return s[:, None] * C


def _build_weights_l1():
    C_T = _dct_matrix(T)
    C_k = _dct_matrix(K)
    W2 = (C_k.T @ C_T[:K, :]).T   # [T, K]
    W2r = W2[::-1, :]
    wce = (W2[:H, :] + W2r[:H, :]) / 2.0   # [H, K] vs e
    wco = (W2[:H, :] - W2r[:H, :]) / 2.0   # [H, K] vs o
    we = np.concatenate([C_T[0:K:2, :H].T, wce], axis=1)   # [H, 461+K]
    wo = np.concatenate([C_T[1:K:2, :H].T, wco], axis=1)   # [H, 461+K]
    return we.astype(np.float32), wo.astype(np.float32)


def _build_weights():
    C_T = _dct_matrix(T)          # [T, T]
    C_trunc = C_T[:K, :]          # [K, T]
    C_k = _dct_matrix(K)          # [K, K]
    W2 = (C_k.T @ C_trunc).T      # [T, K]: x -> x_rec columns
    # level-1 fold of W2 (vs e / o)
    W2r = W2[::-1, :]
    wce = (W2[:H, :] + W2r[:H, :]) / 2.0   # [H, K] vs e
    wco = (W2[:H, :] - W2r[:H, :]) / 2.0   # [H, K] vs o
    # level-2 fold of the e side (vs ee / eo)
    wcer = wce[::-1, :]
    wcee = (wce[:Q, :] + wcer[:Q, :]) / 2.0   # [Q, K] vs ee
    wceo = (wce[:Q, :] - wcer[:Q, :]) / 2.0   # [Q, K] vs eo
    wee = np.concatenate([C_T[0:K:4, :Q].T, wcee], axis=1)   # [Q, NEE+K]
    weo = np.concatenate([C_T[2:K:4, :Q].T, wceo], axis=1)   # [Q, NEO+K]
    wo = np.concatenate([C_T[1:K:2, :H].T, wco], axis=1)     # [H, NODD+K]
    return (wee.astype(np.float32), weo.astype(np.float32),
            wo.astype(np.float32))


PERF_CONTIG_PROBE = bool(os.environ.get("KERNEL_PERF_CONTIG_PROBE"))
# fold level: 1 = e/o only, 2 = ee/eo/o
LEVEL = int(os.environ.get("KERNEL_LEVEL", "1"))
NE1 = 461            # level-1 even dct rows
E1_CHUNKS = _chunks(NE1)


def _build_bass_l1(loop_repeat: int = 1):
    """Level-1 fold: inputs e/o [H], weights we/wo [H, 461+K]. dct even/odd
    chunks are staged pairwise in SBUF so the dct write is contiguous."""
    f32 = mybir.dt.float32
    nc = bacc.Bacc("TRN2", target_bir_lowering=False, debug=False,
                   num_devices=N_CORES)
    e_in = nc.dram_tensor("e", [BPC, H, D], MM_DTYPE,
                          kind="ExternalInput").ap()
    o_in = nc.dram_tensor("o", [BPC, H, D], MM_DTYPE,
                          kind="ExternalInput").ap()
    we_in = nc.dram_tensor("we", [H, NE1 + K], MM_DTYPE,
                           kind="ExternalInput").ap()
    wo_in = nc.dram_tensor("wo", [H, NE1 + K], MM_DTYPE,
                           kind="ExternalInput").ap()
    dct = nc.dram_tensor("dct", [BPC, KPAD, D], f32,
                         kind="ExternalOutput").ap()
    rec = nc.dram_tensor("rec", [BPC, K, D], f32, kind="ExternalOutput").ap()

    dct_p = dct.rearrange("b (k two) d -> b k two d", two=2)
    e_r = e_in.rearrange("b (c p) d -> b p c d", p=P)
    o_r = o_in.rearrange("b (c p) d -> b p c d", p=P)
    we_r = we_in.rearrange("(c p) j -> p c j", p=P)
    wo_r = wo_in.rearrange("(c p) j -> p c j", p=P)

    with tile.TileContext(nc) as tc:
        with (
            tc.tile_pool(name="wp", bufs=1) as wp,
            tc.tile_pool(name="xp", bufs=3) as xp,
            tc.tile_pool(name="op", bufs=6) as op,
            tc.tile_pool(name="pp", bufs=4, space="PSUM") as pp,
        ):
            wet = wp.tile([P, CCO, NE1 + K], MM_DTYPE)
            wot = wp.tile([P, CCO, NE1 + K], MM_DTYPE)
            for (c0, sz) in E1_CHUNKS:
                nc.scalar.dma_start(wet[:, :, c0:c0 + sz],
                                    we_r[:, :, c0:c0 + sz])
            for (c0, sz) in E1_CHUNKS:
                nc.scalar.dma_start(wot[:, :, c0:c0 + sz],
                                    wo_r[:, :, c0:c0 + sz])
            for (c0, sz) in C_CHUNKS:
                nc.scalar.dma_start(wet[:, :, NE1 + c0:NE1 + c0 + sz],
                                    we_r[:, :, NE1 + c0:NE1 + c0 + sz])
                nc.scalar.dma_start(wot[:, :, NE1 + c0:NE1 + c0 + sz],
                                    wo_r[:, :, NE1 + c0:NE1 + c0 + sz])

            def mm_group(pt, wtile, c0, rhs, sz, i, n_mm):
                for cc in range(CCO):
                    st, sp = (i == 0), (i == n_mm - 1)
                    nc.tensor.matmul(
                        pt[:sz, 0:N0], wtile[:, cc, c0:c0 + sz],
                        rhs[:, cc, 0:N0], start=st, stop=sp)
                    nc.tensor.matmul(
                        pt[:sz, N0:D], wtile[:, cc, c0:c0 + sz],
                        rhs[:, cc, N0:D], start=st, stop=sp)
                    i += 1
                return i

            def body():
                for b in range(BPC):
                    et = xp.tile([P, CCO, D], MM_DTYPE, tag="et")
                    ot_in = xp.tile([P, CCO, D], MM_DTYPE, tag="ot_in")
                    nc.sync.dma_start(et[:], e_r[b])
                    nc.sync.dma_start(ot_in[:], o_r[b])

                    for (r0, sz) in E1_CHUNKS:
                        pt_e = pp.tile([P, D], f32, tag="pt")
                        mm_group(pt_e, wet, r0, et, sz, 0, CCO)
                        pt_o = pp.tile([P, D], f32, tag="pt")
                        mm_group(pt_o, wot, r0, ot_in, sz, 0, CCO)
                        so2 = op.tile([P, 2, D], f32, tag="so")
                        nc.vector.tensor_copy(so2[:sz, 0, :], pt_e[:sz, :])
                        nc.vector.tensor_copy(so2[:sz, 1, :], pt_o[:sz, :])
                        nc.sync.dma_start(dct_p[b, r0:r0 + sz], so2[:sz])
                    for (r0, sz) in C_CHUNKS:
                        pt = pp.tile([P, D], f32, tag="pt")
                        i = mm_group(pt, wet, NE1 + r0, et, sz, 0, 2 * CCO)
                        mm_group(pt, wot, NE1 + r0, ot_in, sz, i, 2 * CCO)
                        so = op.tile([P, 2, D], f32, tag="so")
                        nc.vector.tensor_copy(so[:sz, 0, :], pt[:sz, :])
                        nc.sync.dma_start(rec[b, r0:r0 + sz, :],
                                          so[:sz, 0, :])

            if loop_repeat > 1:
                with tc.For_i(0, loop_repeat, 1):
                    body()
            else:
                body()
    nc.compile()
    return nc


def _build_bass_l2(loop_repeat: int = 1):
    """loop_repeat>1 wraps the program in a hardware For_i loop (same
    outputs each trip) — used by test.py for slope-based HW timing."""
    f32 = mybir.dt.float32
    nc = bacc.Bacc("TRN2", target_bir_lowering=False, debug=False,
                   num_devices=N_CORES)
    ee_in = nc.dram_tensor("ee", [BPC, Q, D], MM_DTYPE,
                           kind="ExternalInput").ap()
    eo_in = nc.dram_tensor("eo", [BPC, Q, D], MM_DTYPE,
                           kind="ExternalInput").ap()
    o_in = nc.dram_tensor("o", [BPC, H, D], MM_DTYPE,
                          kind="ExternalInput").ap()
    wee_in = nc.dram_tensor("wee", [Q, NEE + K], MM_DTYPE,
                            kind="ExternalInput").ap()
    weo_in = nc.dram_tensor("weo", [Q, NEO + K], MM_DTYPE,
                            kind="ExternalInput").ap()
    wo_in = nc.dram_tensor("wo", [H, NODD + K], MM_DTYPE,
                           kind="ExternalInput").ap()
    dct = nc.dram_tensor("dct", [BPC, KPAD, D], f32,
                         kind="ExternalOutput").ap()
    rec = nc.dram_tensor("rec", [BPC, K, D], f32, kind="ExternalOutput").ap()

    # dct row views: quads (k%4) and odd pairs
    dct_q = dct.rearrange("b (q four) d -> b four q d", four=4)
    dct_p = dct.rearrange("b (k two) d -> b two k d", two=2)
    ee_r = ee_in.rearrange("b (c p) d -> b p c d", p=P)
    eo_r = eo_in.rearrange("b (c p) d -> b p c d", p=P)
    o_r = o_in.rearrange("b (c p) d -> b p c d", p=P)
    wee_r = wee_in.rearrange("(c p) j -> p c j", p=P)
    weo_r = weo_in.rearrange("(c p) j -> p c j", p=P)
    wo_r = wo_in.rearrange("(c p) j -> p c j", p=P)

    with tile.TileContext(nc) as tc:
        with (
            tc.tile_pool(name="wp", bufs=1) as wp,
            tc.tile_pool(name="xp", bufs=3) as xp,
            tc.tile_pool(name="op", bufs=6) as op,
            tc.tile_pool(name="pp", bufs=4, space="PSUM") as pp,
        ):
            weet = wp.tile([P, CCE, NEE + K], MM_DTYPE)
            weot = wp.tile([P, CCE, NEO + K], MM_DTYPE)
            wot = wp.tile([P, CCO, NODD + K], MM_DTYPE)

            # Weights stream on the ACT HWDGE ring (nc.scalar) in batch-0
            # consumption order; inputs/outputs use the SP ring (nc.sync).
            for (c0, sz) in EE_CHUNKS:
                nc.scalar.dma_start(weet[:, :, c0:c0 + sz],
                                    wee_r[:, :, c0:c0 + sz])
            for (c0, sz) in EO_CHUNKS:
                nc.scalar.dma_start(weot[:, :, c0:c0 + sz],
                                    weo_r[:, :, c0:c0 + sz])
            for (c0, sz) in O_CHUNKS:
                nc.scalar.dma_start(wot[:, :, c0:c0 + sz],
                                    wo_r[:, :, c0:c0 + sz])
            for (c0, sz) in C_CHUNKS:
                nc.scalar.dma_start(weet[:, :, NEE + c0:NEE + c0 + sz],
                                    wee_r[:, :, NEE + c0:NEE + c0 + sz])
                nc.scalar.dma_start(weot[:, :, NEO + c0:NEO + c0 + sz],
                                    weo_r[:, :, NEO + c0:NEO + c0 + sz])
                nc.scalar.dma_start(wot[:, :, NODD + c0:NODD + c0 + sz],
                                    wo_r[:, :, NODD + c0:NODD + c0 + sz])

            def mm_group(pt, wtile, ncc, c0, rhs, sz, i, n_mm):
                for cc in range(ncc):
                    st, sp = (i == 0), (i == n_mm - 1)
                    nc.tensor.matmul(
                        pt[:sz, 0:N0], wtile[:, cc, c0:c0 + sz],
                        rhs[:, cc, 0:N0], start=st, stop=sp)
                    nc.tensor.matmul(
                        pt[:sz, N0:D], wtile[:, cc, c0:c0 + sz],
                        rhs[:, cc, N0:D], start=st, stop=sp)
                    i += 1
                return i

            def emit(groups, dest_ap, sz):
                pt = pp.tile([P, D], f32, tag="pt")
                n_mm = sum(g[2] for g in groups)
                i = 0
                for (wtile, c0, ncc, rhs) in groups:
                    i = mm_group(pt, wtile, ncc, c0, rhs, sz, i, n_mm)
                so = op.tile([P, D], f32, tag="so")
                nc.vector.tensor_copy(so[:sz, :], pt[:sz, :])
                nc.sync.dma_start(dest_ap, so[:sz, :])

            def body():
                for b in range(BPC):
                    eet = xp.tile([P, CCE, D], MM_DTYPE, tag="eet")
                    eot = xp.tile([P, CCE, D], MM_DTYPE, tag="eot")
                    ot = xp.tile([P, CCO, D], MM_DTYPE, tag="ot")
                    nc.sync.dma_start(eet[:], ee_r[b])
                    nc.sync.dma_start(eot[:], eo_r[b])
                    nc.sync.dma_start(ot[:], o_r[b])

                    if PERF_CONTIG_PROBE:  # timing probe: contiguous writes
                        for (r0, sz) in EE_CHUNKS:
                            emit([(weet, r0, CCE, eet)],
                                 dct[b, r0:r0 + sz, :], sz)
                        for (r0, sz) in EO_CHUNKS:
                            emit([(weot, r0, CCE, eot)],
                                 dct[b, 256 + r0:256 + r0 + sz, :], sz)
                        for (r0, sz) in O_CHUNKS:
                            emit([(wot, r0, CCO, ot)],
                                 dct[b, 464 + r0:464 + r0 + sz, :], sz)
                    else:
                        for (r0, sz) in EE_CHUNKS:   # dct rows 4i
                            emit([(weet, r0, CCE, eet)],
                                 dct_q[b, 0, r0:r0 + sz, :], sz)
                        for (r0, sz) in EO_CHUNKS:   # dct rows 4i+2
                            emit([(weot, r0, CCE, eot)],
                                 dct_q[b, 2, r0:r0 + sz, :], sz)
                        for (r0, sz) in O_CHUNKS:    # dct rows 2j+1
                            emit([(wot, r0, CCO, ot)],
                                 dct_p[b, 1, r0:r0 + sz, :], sz)
                    for (r0, sz) in C_CHUNKS:    # rec rows: ee+eo+o parts
                        emit([(weet, NEE + r0, CCE, eet),
                              (weot, NEO + r0, CCE, eot),
                              (wot, NODD + r0, CCO, ot)],
                             rec[b, r0:r0 + sz, :], sz)

            if loop_repeat > 1:
                with tc.For_i(0, loop_repeat, 1):
                    body()
            else:
                body()
    nc.compile()
    return nc


def _build_bass(loop_repeat: int = 1):
    if LEVEL == 1:
        return _build_bass_l1(loop_repeat)
    return _build_bass_l2(loop_repeat)


_CACHE = {}


def _get():
    if "nc" not in _CACHE:
        _CACHE["nc"] = _build_bass()
        _CACHE["w"] = (_build_weights_l1() if LEVEL == 1
                       else _build_weights())
    return _CACHE["nc"], _CACHE["w"]


def _fold(x: np.ndarray):
    """x [b, T, D] -> ee, eo [b, Q, D], o [b, H, D] (mirror folds)."""
    lo = x[:, :H, :]
    hi = x[:, :H - 1:-1, :]
    e = lo + hi
    o = lo - hi
    ee = e[:, :Q, :] + e[:, :Q - 1:-1, :]
    eo = e[:, :Q, :] - e[:, :Q - 1:-1, :]
    return ee, eo, o


def _make_in_maps(x: np.ndarray):
    _, w = _get()
    x = np.ascontiguousarray(x, dtype=np.float32)
    if LEVEL == 1:
        we, wo = w
        lo = x[:, :H, :]
        hi = x[:, :H - 1:-1, :]
        e = np.ascontiguousarray(lo + hi)
        o = np.ascontiguousarray(lo - hi)
        return [
            {"e": e[c * BPC:(c + 1) * BPC], "o": o[c * BPC:(c + 1) * BPC],
             "we": we, "wo": wo}
            for c in range(N_CORES)
        ]
    wee, weo, wo = w
    ee, eo, o = _fold(x)
    ee = np.ascontiguousarray(ee)
    eo = np.ascontiguousarray(eo)
    o = np.ascontiguousarray(o)
    return [
        {"ee": ee[c * BPC:(c + 1) * BPC], "eo": eo[c * BPC:(c + 1) * BPC],
         "o": o[c * BPC:(c + 1) * BPC], "wee": wee, "weo": weo, "wo": wo}
        for c in range(N_CORES)
    ]


def kernel(x: np.ndarray, _results_out=None):
    """x [64, 1024, 768] fp32 -> (x_rec [64, 922, 768], x_dct [64, 922, 768])."""
    nc, _ = _get()
    in_maps = _make_in_maps(x)
    res = run_bass_kernel_spmd(nc, in_maps, core_ids=list(range(N_CORES)))
    if _results_out is not None:
        _results_out.append(res)
    x_rec = np.concatenate([r["rec"] for r in res.results], axis=0)
    x_dct = np.concatenate([r["dct"][:, :K, :] for r in res.results], axis=0)
    return x_rec, x_dct



# revision 4
# speedup vs baseline: 1.4462x; 1.4462x over previous
"""DCT sequence-compression kernel for TRN2 (nn_CompressedModel).

For x [B=64, T=1024, D=768] fp32 computes (matching the reference):
  x_dct = (C_T @ x)[:, :k, :]          k = 922
  x_rec = C_k^T @ x_dct
returning (x_rec, x_dct).

Two mirror symmetries cut the tensor-engine work 3x vs the naive pair
of matmuls:

1) Input fold (host-side): C[k, T-1-t] = (-1)^k C[k, t], so with
   e = x[:512] + rev(x[512:]), o = x[:512] - rev(x[512:]) the even dct
   rows contract only e and the odd rows only o (512-long contractions).

2) Output fold (new): P = C_k^T C_trunc satisfies
   P[K-1-n, t] = P[n, T-1-t], which makes the folded reconstruction
   weights wce/wco column-(anti)symmetric: wce[:, K-1-n] = wce[:, n],
   wco[:, K-1-n] = -wco[:, n]. Hence with
     s = wce[:, :461]^T e   (only e!)
     d = wco[:, :461]^T o   (only o!)
   rec[n] = s[n] + d[n] and rec[K-1-n] = s[n] - d[n] for n < 461.
   Reconstruction streams half the columns; the combine is two cheap
   vector adds out of PSUM.

All matmuls run in bf16 (same 1 col/cycle PE rate as float32r on TRN2,
but half the HBM traffic); accumulation stays fp32 in PSUM, outputs are
written bf16 and upcast on host. Measured end-to-end rel err ~3e-3,
within the 2e-2 gate. Pure data parallel over B across 8 cores.
"""

import os

import ml_dtypes
import numpy as np

# The trimmed axon environment has no NTFF profile hook; make sure
# run_bass_kernel_spmd never tries the trace path.
os.environ["BASS_NEVER_TRACE"] = "1"

import concourse.bass as bass  # noqa: F401
import concourse.mybir as mybir
import concourse.tile as tile
from concourse import bacc
from concourse.bass_utils import run_bass_kernel_spmd

B, T, D = 64, 1024, 768
K = 922              # ceil(0.9 * 1024)
KH = K // 2          # 461: folded output rows
H = T // 2           # 512: contraction length
N_CORES = 8
BPC = B // N_CORES   # batches per core
P = 128
CC = H // P          # 4 contraction chunks
N0 = 512             # first free-dim split (PSUM bank)

BF16 = mybir.dt.bfloat16
NPBF16 = ml_dtypes.bfloat16


def _chunks(n, p=P):
    return [(i * p, min(p, n - i * p)) for i in range((n + p - 1) // p)]


R_CHUNKS = _chunks(KH)   # 4 chunks: 128,128,128,77


def _dct_matrix(N: int) -> np.ndarray:
    """Orthonormal DCT-II matrix [N, N] in float64."""
    n = np.arange(N, dtype=np.float64)
    C = np.cos(np.pi * (2.0 * n[None, :] + 1.0) * n[:, None] / (2.0 * N))
    s = np.full(N, np.sqrt(2.0 / N))
    s[0] = np.sqrt(1.0 / N)
    return s[:, None] * C


def _build_weights():
    C_T = _dct_matrix(T)
    C_k = _dct_matrix(K)
    W2 = (C_k.T @ C_T[:K, :]).T            # [T, K]: x -> x_rec columns
    W2r = W2[::-1, :]
    wce = (W2[:H, :] + W2r[:H, :]) / 2.0   # [H, K] vs e; cols mirror-sym
    wco = (W2[:H, :] - W2r[:H, :]) / 2.0   # [H, K] vs o; cols mirror-anti
    we = np.concatenate([C_T[0:K:2, :H].T, wce[:, :KH]], axis=1)  # [H, 922]
    wo = np.concatenate([C_T[1:K:2, :H].T, wco[:, :KH]], axis=1)  # [H, 922]
    return we.astype(NPBF16), wo.astype(NPBF16)


def _build_bass(loop_repeat: int = 1):
    """loop_repeat>1 wraps the program in a hardware For_i loop (same
    outputs each trip) — used by test.py for slope-based HW timing."""
    f32 = mybir.dt.float32
    nc = bacc.Bacc("TRN2", target_bir_lowering=False, debug=False,
                   num_devices=N_CORES)
    e_in = nc.dram_tensor("e", [BPC, H, D], BF16, kind="ExternalInput").ap()
    o_in = nc.dram_tensor("o", [BPC, H, D], BF16, kind="ExternalInput").ap()
    we_in = nc.dram_tensor("we", [H, 2 * KH], BF16,
                           kind="ExternalInput").ap()
    wo_in = nc.dram_tensor("wo", [H, 2 * KH], BF16,
                           kind="ExternalInput").ap()
    dct = nc.dram_tensor("dct", [BPC, K, D], BF16,
                         kind="ExternalOutput").ap()
    # rec2[:, :, 0] = rec rows 0..460; rec2[:, :, 1, n] = rec row 921-n
    rec2 = nc.dram_tensor("rec2", [BPC, KH, 2, D], BF16,
                          kind="ExternalOutput").ap()

    dct_p = dct.rearrange("b (k two) d -> b k two d", two=2)
    e_r = e_in.rearrange("b (c p) d -> b p c d", p=P)
    o_r = o_in.rearrange("b (c p) d -> b p c d", p=P)
    we_r = we_in.rearrange("(c p) j -> p c j", p=P)
    wo_r = wo_in.rearrange("(c p) j -> p c j", p=P)

    with tile.TileContext(nc) as tc:
        with (
            tc.tile_pool(name="wp", bufs=1) as wp,
            tc.tile_pool(name="xp", bufs=3) as xp,
            tc.tile_pool(name="op", bufs=6) as op,
            tc.tile_pool(name="pp", bufs=4, space="PSUM") as pp,
        ):
            wet = wp.tile([P, CC, 2 * KH], BF16)
            wot = wp.tile([P, CC, 2 * KH], BF16)
            # Weights stream on the ACT HWDGE ring in consumption order;
            # inputs/outputs use the SP ring.
            for (c0, sz) in R_CHUNKS:
                nc.scalar.dma_start(wet[:, :, c0:c0 + sz],
                                    we_r[:, :, c0:c0 + sz])
                nc.scalar.dma_start(wot[:, :, c0:c0 + sz],
                                    wo_r[:, :, c0:c0 + sz])
            for (c0, sz) in R_CHUNKS:
                nc.scalar.dma_start(wet[:, :, KH + c0:KH + c0 + sz],
                                    we_r[:, :, KH + c0:KH + c0 + sz])
                nc.scalar.dma_start(wot[:, :, KH + c0:KH + c0 + sz],
                                    wo_r[:, :, KH + c0:KH + c0 + sz])

            def mm_group(pt, wtile, c0, rhs, sz):
                for cc in range(CC):
                    st, sp = (cc == 0), (cc == CC - 1)
                    nc.tensor.matmul(
                        pt[:sz, 0:N0], wtile[:, cc, c0:c0 + sz],
                        rhs[:, cc, 0:N0], start=st, stop=sp)
                    nc.tensor.matmul(
                        pt[:sz, N0:D], wtile[:, cc, c0:c0 + sz],
                        rhs[:, cc, N0:D], start=st, stop=sp)

            def body():
                for b in range(BPC):
                    et = xp.tile([P, CC, D], BF16, tag="et")
                    ot = xp.tile([P, CC, D], BF16, tag="ot")
                    nc.sync.dma_start(et[:], e_r[b])
                    nc.sync.dma_start(ot[:], o_r[b])

                    for (r0, sz) in R_CHUNKS:
                        # dct rows 2n (from e) and 2n+1 (from o)
                        pt_e = pp.tile([P, D], f32, tag="pt")
                        mm_group(pt_e, wet, r0, et, sz)
                        pt_o = pp.tile([P, D], f32, tag="pt")
                        mm_group(pt_o, wot, r0, ot, sz)
                        so2 = op.tile([P, 2, D], BF16, tag="so")
                        nc.scalar.copy(so2[:sz, 0, :], pt_e[:sz, :])
                        nc.scalar.copy(so2[:sz, 1, :], pt_o[:sz, :])
                        nc.sync.dma_start(dct_p[b, r0:r0 + sz], so2[:sz])

                        # rec halves: s (from e) and d (from o). The DVE
                        # can read only one PSUM operand per tensor_tensor,
                        # so s is staged through SBUF by the ACT engine.
                        pt_s = pp.tile([P, D], f32, tag="pt")
                        mm_group(pt_s, wet, KH + r0, et, sz)
                        pt_d = pp.tile([P, D], f32, tag="pt")
                        mm_group(pt_d, wot, KH + r0, ot, sz)
                        st = op.tile([P, D], f32, tag="st")
                        nc.scalar.copy(st[:sz, :], pt_s[:sz, :])
                        sr = op.tile([P, 2, D], BF16, tag="sr")
                        nc.vector.tensor_add(sr[:sz, 0, :], pt_d[:sz, :],
                                             st[:sz, :])
                        nc.vector.tensor_sub(sr[:sz, 1, :], st[:sz, :],
                                             pt_d[:sz, :])
                        nc.sync.dma_start(rec2[b, r0:r0 + sz], sr[:sz])

            if loop_repeat > 1:
                with tc.For_i(0, loop_repeat, 1):
                    body()
            else:
                body()
    nc.compile()
    return nc


_CACHE = {}


def _get():
    if "nc" not in _CACHE:
        _CACHE["nc"] = _build_bass()
        _CACHE["w"] = _build_weights()
    return _CACHE["nc"], _CACHE["w"]


def _make_in_maps(x: np.ndarray):
    _, w = _get()
    we, wo = w
    x = np.asarray(x, dtype=np.float32)
    lo = x[:, :H, :]
    hi = x[:, :H - 1:-1, :]
    e = np.ascontiguousarray(lo + hi, dtype=NPBF16)
    o = np.ascontiguousarray(lo - hi, dtype=NPBF16)
    return [
        {"e": e[c * BPC:(c + 1) * BPC], "o": o[c * BPC:(c + 1) * BPC],
         "we": we, "wo": wo}
        for c in range(N_CORES)
    ]


def kernel(x: np.ndarray, _results_out=None):
    """x [64, 1024, 768] fp32 -> (x_rec [64, 922, 768], x_dct [64, 922, 768])."""
    nc, _ = _get()
    in_maps = _make_in_maps(x)
    res = run_bass_kernel_spmd(nc, in_maps, core_ids=list(range(N_CORES)))
    if _results_out is not None:
        _results_out.append(res)
    x_dct = np.concatenate(
        [np.asarray(r["dct"]) for r in res.results], axis=0
    ).astype(np.float32)
    rec_parts = []
    for r in res.results:
        r2 = np.asarray(r["rec2"])                      # [BPC, 461, 2, D]
        lo = r2[:, :, 0, :]
        hi = r2[:, ::-1, 1, :]                          # rec rows 461..921
        rec_parts.append(np.concatenate([lo, hi], axis=1))
    x_rec = np.concatenate(rec_parts, axis=0).astype(np.float32)
    return x_rec, x_dct
